# revision 49
# baseline (speedup 1.0000x reference)
"""DeepSeekV3 MLA attention prefill kernel for 8 Trainium2 NeuronCores.

Sharding: sequence-parallel for the low-rank input projections (q_a / kv_a),
AllGather of the shared latents, tensor-parallel over heads (4 heads/core) for
q_b / kv_b decompression and attention, AllGather of attention outputs, and
output-column-parallel o_proj (each core produces a disjoint 512-column slice
of the output, concatenated on host).

v3: float16 operands, host-packed contiguous weight tiles, host-precomputed
rope tables, causal mask folded into the score matmul, XBAR DMA-transpose for
the latent gather (no PE transposes), decompress-input loads on the Act DMA
queue so decompression starts the moment the PE frees up, per-q-block softmax
reciprocals with head-normalization deferred into the next phase, and o_proj
groups scheduled h0 h1 h2 g0 h3 g1 g2 g3 so every AllGather hides.
"""

import sys

sys.path.insert(0, "/opt/trn_rl_repo")

import numpy as np

import concourse.bass as bass  # noqa: F401
import concourse.mybir as mybir
from concourse import bacc
from concourse.bass import ds, ts
from concourse.tile import TileContext
from concourse.bass_utils import run_bass_kernel_spmd
from contextlib import ExitStack

F = mybir.dt.float32
F16 = mybir.dt.float16
R = mybir.dt.float32r
AF = mybir.ActivationFunctionType
ALU = mybir.AluOpType

NCORES = 8
B, S, H = 1, 2048, 4096
N_HEADS = 32
HPC = N_HEADS // NCORES          # heads per core = 4
SL = S // NCORES                 # sequence rows per core = 256
QR, KR = 1536, 512
DR, DN, DV = 64, 128, 128
QD = DN + DR                     # 192
SCALE = QD ** -0.5
EPS = 1e-6
THETA = 10000.0
MCOLS = H // NCORES              # output columns per core = 512
NKT = H // 128                   # 32 k-tiles over the model dim
NR = QR // 128                   # 12 k-tiles over q_lora_rank
NKR = KR // 128                  # 4 k-tiles over kv_lora_rank
KD = KR + DR                     # 576
MASKV = -30000.0

LAST_RESULT = None               # test harness reads exec_time_ns from here
_CACHED_NC = None


def _build_program():
    nc = bacc.Bacc(None, target_bir_lowering=False, num_devices=NCORES)

    # ---------------- DRAM declarations ----------------
    xP = nc.dram_tensor("xP", [128, NKT * SL], F16, kind="ExternalInput")
    qawP = nc.dram_tensor("qawP", [12, 128, 8 * 512], F16, kind="ExternalInput")
    kvawP = nc.dram_tensor("kvawP", [4, 128, 8 * 576], F16, kind="ExternalInput")
    qbwP = nc.dram_tensor("qbwP", [NR, 128, 1024], F16, kind="ExternalInput")
    kvbwP = nc.dram_tensor("kvbwP", [NKR, 128, 1024], F16, kind="ExternalInput")
    owP = nc.dram_tensor("owP", [N_HEADS * DV, MCOLS], F16, kind="ExternalInput")
    cosqP = nc.dram_tensor("cosqP", [128, S], F, kind="ExternalInput")
    sinqP = nc.dram_tensor("sinqP", [128, S], F, kind="ExternalInput")
    coskP = nc.dram_tensor("coskP", [2, 128, 32], F, kind="ExternalInput")
    sinkP = nc.dram_tensor("sinkP", [2, 128, 32], F, kind="ExternalInput")
    identH = nc.dram_tensor("identH", [128, 128], F16, kind="ExternalInput")
    maskTP = nc.dram_tensor("maskTP", [128, 128], F16, kind="ExternalInput")
    selP = nc.dram_tensor("selP", [4, 128, 128], F, kind="ExternalInput")
    out_d = nc.dram_tensor("out", [MCOLS, S], F, kind="ExternalOutput")

    # latents gathered in [token, feature] layout; consumers transpose-read
    g1kv_src = nc.dram_tensor("g1kv_src", [SL, KD], F16)
    g1kv = nc.dram_tensor("g1kv", [NCORES, SL, KD], F16, addr_space="Shared")
    g1q_src = nc.dram_tensor("g1q_src", [SL, QR], F16)
    g1q = nc.dram_tensor("g1q", [NCORES, SL, QR], F16, addr_space="Shared")
    g2_src = [nc.dram_tensor(f"g2src{h}", [DV, S], F16) for h in range(HPC)]
    g2 = [
        nc.dram_tensor(f"g2_{h}", [NCORES, DV, S], F16, addr_space="Shared")
        for h in range(HPC)
    ]
    RG = [list(range(NCORES))]
    g1kv_flat = g1kv.ap().rearrange("c s f -> (c s) f")
    g1q_flat = g1q.ap().rearrange("c s f -> (c s) f")

    with TileContext(nc) as tc, ExitStack() as ctx:
        persist = ctx.enter_context(tc.tile_pool(name="persist", bufs=1))
        ckvp = ctx.enter_context(tc.tile_pool(name="ckvp", bufs=1))

        # warm up the GPSIMD Q7 cores early: the first collective otherwise
        # pays ~15us of library-load latency on the critical path
        warm = persist.tile([128, 4], F, name="c_warm")
        nc.gpsimd.memset(warm[:], 0.0)

        # input activations + first weight tiles first — the first matmul
        # waits only on chunk 0; constants follow (not needed until rms)
        xtp = ExitStack()
        xtpool = xtp.enter_context(tc.tile_pool(name="xtp", bufs=1))
        xt = xtpool.tile([128, NKT, SL], F16, name="xt")
        for g in range(4):
            nc.sync.dma_start(
                xt[:, ds(8 * g, 8), :],
                xP.ap()[:, ds(8 * g * SL, 8 * SL)].rearrange(
                    "p (kt s) -> p kt s", kt=8
                ),
            )
        wp0_e = ExitStack()
        wp0 = wp0_e.enter_context(tc.tile_pool(name="wp0", bufs=4))
        kvw_tiles = []
        for ktg in range(4):
            w = wp0.tile([128, 8, 576], F16, tag="kvw", name=f"kvw{ktg}")
            nc.sync.dma_start(
                w[:], kvawP.ap()[ktg].rearrange("p (k n) -> p k n", k=8)
            )
            kvw_tiles.append(w)

        # ---------------- constants ----------------
        identh = persist.tile([128, 128], F16, name="c_identh")
        nc.sync.dma_start(identh[:], identH[:])
        maskT = persist.tile([128, 128], F16, name="c_maskT")
        nc.sync.dma_start(maskT[:], maskTP[:])
        ones_col = persist.tile([128, 1], F16, name="c_ones_col")
        nc.vector.memset(ones_col[:], 1.0)
        sel4 = []
        for qb in range(4):
            s = persist.tile([128, 128], R, name=f"c_sel{qb}")
            nc.sync.dma_start(s[:], selP.ap()[qb].bitcast(R))
            sel4.append(s)
        eps_t = persist.tile([128, 1], F, name="c_eps")
        nc.vector.memset(eps_t[:], EPS)
        sin_k = [persist.tile([128, 32], F, name=f"t_sink{st}") for st in range(2)]
        cos_k = [persist.tile([128, 32], F, name=f"t_cosk{st}") for st in range(2)]
        for st in range(2):
            nc.sync.dma_start(sin_k[st][:], sinkP.ap()[st])
            nc.sync.dma_start(cos_k[st][:], coskP.ap()[st])

        # persistent attention-phase state
        kx = persist.tile([128, S], F16, name="kx")        # [64:128] = k_pe
        kpe0 = persist.tile([64, S], F16, name="kpe0")     # k_pe for even heads
        qnope = [persist.tile([128, S], F16, name=f"qnope{h}") for h in range(HPC)]
        qfpe = [persist.tile([128, S], F16, name=f"qfpe{p}") for p in range(2)]
        kn_all = [persist.tile([128, S], F16, name=f"kn{h}") for h in range(HPC)]
        v_sb = persist.tile([128, S // 128, 512], F16, name="v_sb")

        # ---------------- phase 0: q_a / kv_a projections ----------------
        ctx0 = ExitStack()
        p0 = ctx0.enter_context(tc.tile_pool(name="p0", bufs=2))

        # --- kv_a: 512-col ckv chunk + 64-col k_pe chunk per st ---
        ctx0a = ExitStack()
        kv0_ps = ctx0a.enter_context(tc.tile_pool(name="kv0_ps", bufs=2, space="PSUM"))
        kv1_ps = ctx0a.enter_context(tc.tile_pool(name="kv1_ps", bufs=2, space="PSUM"))

        kvch = [
            kv0_ps.tile([128, 512], F, tag="kv0", name=f"kvps{st}") for st in range(2)
        ]
        kvpe = [
            kv1_ps.tile([128, 64], F, tag="kv1", name=f"kvpe{st}") for st in range(2)
        ]
        for ktg in range(4):
            w = kvw_tiles[ktg]
            for kk in range(8):
                kt = ktg * 8 + kk
                for st in range(2):
                    nc.tensor.matmul(
                        kvch[st][:], xt[:, kt, ts(st, 128)], w[:, kk, 0:512],
                        start=(kt == 0), stop=(kt == NKT - 1),
                    )
                    nc.tensor.matmul(
                        kvpe[st][:], xt[:, kt, ts(st, 128)], w[:, kk, 512:576],
                        start=(kt == 0), stop=(kt == NKT - 1),
                    )
        for st in range(2):
            acc0 = p0.tile([128, 1], F, tag="kvacc", name=f"kvacc{st}")
            scr = p0.tile([128, 512], F, tag="kvscr", name=f"kvscr{st}")
            nc.scalar.activation(scr[:], kvch[st][:], AF.Square, accum_out=acc0[:])
            stdv = p0.tile([128, 1], F, tag="kvstd", name=f"kvstd{st}")
            nc.scalar.activation(
                stdv[:], acc0[:], AF.Sqrt, bias=eps_t[:], scale=1.0 / KR
            )
            rinv = p0.tile([128, 1], F, tag="kvrinv", name=f"kvrinv{st}")
            nc.vector.reciprocal(rinv[:], stdv[:])
            ckvn = p0.tile([128, KR], F16, tag="ckvn", name=f"ckvn{st}")
            nc.vector.tensor_scalar_mul(ckvn[:], kvch[st][:], rinv[:])
            nc.sync.dma_start(g1kv_src.ap()[ts(st, 128), 0:KR], ckvn[:])
            # rope k_pe: deinterleave even/odd from the 64-col chunk
            pe = kvpe[st][:].rearrange("p (d two) -> p two d", two=2)
            y1, y2 = pe[:, 0], pe[:, 1]
            kr_t = p0.tile([128, DR], F16, tag="kr", name=f"kr{st}")
            t1 = p0.tile([128, 32], F, tag="krt1", name=f"krt1_{st}")
            t2 = p0.tile([128, 32], F, tag="krt2", name=f"krt2_{st}")
            nc.vector.tensor_tensor(t1[:], y1, cos_k[st][:], ALU.mult)
            nc.vector.tensor_tensor(t2[:], y2, sin_k[st][:], ALU.mult)
            nc.vector.tensor_tensor(kr_t[:, 0:32], t1[:], t2[:], ALU.subtract)
            nc.vector.tensor_tensor(t1[:], y2, cos_k[st][:], ALU.mult)
            nc.vector.tensor_tensor(t2[:], y1, sin_k[st][:], ALU.mult)
            nc.vector.tensor_tensor(kr_t[:, 32:64], t1[:], t2[:], ALU.add)
            nc.sync.dma_start(g1kv_src.ap()[ts(st, 128), KR:KD], kr_t[:])
        ctx0a.close()

        nc.gpsimd.collective_compute(
            "AllGather", ALU.bypass,
            ins=[g1kv_src.ap().opt()], outs=[g1kv.ap().opt()], replica_groups=RG,
        )

        # --- q_a (3 chunks of 512 cols) ---
        ctx0b = ExitStack()
        qa_ps = ctx0b.enter_context(tc.tile_pool(name="qa_ps", bufs=6, space="PSUM"))
        qch = [[None] * 3 for _ in range(2)]
        for ch in range(3):
            for st in range(2):
                qch[st][ch] = qa_ps.tile(
                    [128, 512], F, tag="qa_ps", name=f"qaps{st}_{ch}"
                )
        for ch in range(3):
            for ktg in range(4):
                w = wp0.tile([128, 8, 512], F16, tag="qaw", name=f"qaw{ch}_{ktg}")
                nc.sync.dma_start(
                    w[:], qawP.ap()[ch * 4 + ktg].rearrange("p (k n) -> p k n", k=8)
                )
                for kk in range(8):
                    kt = ktg * 8 + kk
                    for st in range(2):
                        nc.tensor.matmul(
                            qch[st][ch][:], xt[:, kt, ts(st, 128)], w[:, kk],
                            start=(kt == 0), stop=(kt == NKT - 1),
                        )
        for st in range(2):
            accs = []
            scr = p0.tile([128, 512], F, tag="qascr", name=f"qascr{st}")
            for ch in range(3):
                a = p0.tile([128, 1], F, tag="qaacc", name=f"qaacc{st}_{ch}")
                nc.scalar.activation(scr[:], qch[st][ch][:], AF.Square, accum_out=a[:])
                accs.append(a)
            nc.vector.tensor_tensor(accs[0][:], accs[0][:], accs[1][:], ALU.add)
            nc.vector.tensor_tensor(accs[0][:], accs[0][:], accs[2][:], ALU.add)
            stdv = p0.tile([128, 1], F, tag="qastd", name=f"qastd{st}")
            nc.scalar.activation(
                stdv[:], accs[0][:], AF.Sqrt, bias=eps_t[:], scale=1.0 / QR
            )
            rinv = p0.tile([128, 1], F, tag="qarinv", name=f"qarinv{st}")
            nc.vector.reciprocal(rinv[:], stdv[:])
            qn = p0.tile([128, QR], F16, tag="qn", name=f"qn{st}")
            for ch in range(3):
                nc.vector.tensor_scalar_mul(qn[:, ts(ch, 512)], qch[st][ch][:], rinv[:])
            nc.sync.dma_start(g1q_src.ap()[ts(st, 128), :], qn[:])
        ctx0b.close()
        ctx0.close()
        wp0_e.close()
        xtp.close()

        nc.gpsimd.collective_compute(
            "AllGather", ALU.bypass,
            ins=[g1q_src.ap().opt()], outs=[g1q.ap().opt()], replica_groups=RG,
        )

        # decompress inputs (transpose-read of the gathered kv latent)
        kvbw = []
        for r in range(NKR):
            t = ckvp.tile([128, 1024], F16, name=f"kvbw{r}")
            nc.sync.dma_start(t[:], kvbwP.ap()[r])
            kvbw.append(t)
        ckv_t = []
        for r in range(NKR):
            t = ckvp.tile([128, S], F16, name=f"ckv{r}")
            nc.sync.dma_start_transpose(t[:], g1kv_flat[:, ts(r, 128)])
            ckv_t.append(t)
        nc.sync.dma_start_transpose(kx[:], g1kv_flat[:, KD - 128 : KD])
        nc.sync.dma_start(kpe0[:], kx[ds(64, 64), :])

        # o_proj weights / output accumulators (space freed by phase 0)
        persist2 = ctx.enter_context(tc.tile_pool(name="persist2", bufs=1))
        osb = [persist2.tile([128, 4, 512], F, name=f"osb{mt}") for mt in range(4)]
        ow_t = [
            [persist2.tile([128, MCOLS], F16, name=f"ow{g}_{r8}")
             for r8 in range(NCORES)]
            for g in range(HPC)
        ]

        # ---------------- decompression (overlaps the q AllGather) ----------
        dec_ctx = ExitStack()
        dec_ps = dec_ctx.enter_context(tc.tile_pool(name="dec_ps", bufs=4, space="PSUM"))
        for st in range(S // 128):
            ps = dec_ps.tile([128, 512], F, tag="dec", name=f"vps{st}")
            for r in range(NKR):
                nc.tensor.matmul(
                    ps[:], ckv_t[r][:, ts(st, 128)], kvbw[r][:, 512:1024],
                    start=(r == 0), stop=(r == NKR - 1),
                )
            nc.vector.tensor_copy(v_sb[:, st, :], ps[:])
        for h in range(HPC):
            for sb in range(4):
                ps = dec_ps.tile([128, 512], F, tag="dec", name=f"knps{h}_{sb}")
                for r in range(NKR):
                    nc.tensor.matmul(
                        ps[:], kvbw[r][:, ts(h, 128)], ckv_t[r][:, ts(sb, 512)],
                        start=(r == 0), stop=(r == NKR - 1),
                    )
                nc.scalar.copy(kn_all[h][:, ts(sb, 512)], ps[:])
        dec_ctx.close()

        # ---------------- q_b projection (+ q rope) ----------------
        ctx2 = ExitStack()
        qbwp = ctx2.enter_context(tc.tile_pool(name="qbwp", bufs=1))
        qrp = ctx2.enter_context(tc.tile_pool(name="qrp", bufs=12))
        ropep = ctx2.enter_context(tc.tile_pool(name="ropep", bufs=1))
        sinp = ctx2.enter_context(tc.tile_pool(name="sinp", bufs=2))
        qb_ps = ctx2.enter_context(tc.tile_pool(name="qb_ps", bufs=8, space="PSUM"))

        qbw = []
        for r in range(NR):
            t = qbwp.tile([128, 1024], F16, name=f"qbw{r}")
            nc.sync.dma_start(t[:], qbwP.ap()[r])
            qbw.append(t)

        for sb in range(4):
            sin_q = sinp.tile([128, 512], F, tag="sinq", name=f"t_sinq{sb}")
            cos_q = sinp.tile([128, 512], F, tag="cosq", name=f"t_cosq{sb}")
            nc.sync.dma_start(sin_q[:], sinqP.ap()[:, ts(sb, 512)])
            nc.sync.dma_start(cos_q[:], cosqP.ap()[:, ts(sb, 512)])
            qr_tiles = []
            for r in range(NR):
                t = qrp.tile([128, 512], F16, tag="qr", name=f"qr{sb}_{r}")
                nc.sync.dma_start_transpose(
                    t[:], g1q_flat[ds(2 * sb * SL, 2 * SL), ts(r, 128)]
                )
                qr_tiles.append(t)
            rope_t1 = {}
            for j in range(8):
                psj = qb_ps.tile([128, 512], F, tag="qb_ps", name=f"qbps{sb}_{j}")
                for r in range(NR):
                    nc.tensor.matmul(
                        psj[:], qbw[r][:, ts(j, 128)], qr_tiles[r][:],
                        start=(r == 0), stop=(r == NR - 1),
                    )
                if j < 4:
                    nc.scalar.copy(qnope[j][:, ts(sb, 512)], psj[:])
                elif j < 6:
                    p = j - 4
                    t1 = ropep.tile([128, 512], F, tag=f"rope1_{p}", name=f"rp1_{sb}_{p}")
                    nc.vector.tensor_tensor(t1[:], psj[:], cos_q[:], ALU.mult)
                    rope_t1[p] = t1
                else:
                    p = j - 6
                    t2 = ropep.tile([128, 512], F, tag=f"rope2_{p}", name=f"rp2_{sb}_{p}")
                    nc.vector.tensor_tensor(t2[:], psj[:], sin_q[:], ALU.mult)
                    nc.vector.tensor_tensor(
                        qfpe[p][:, ts(sb, 512)], rope_t1[p][:], t2[:], ALU.add
                    )
        ctx2.close()

        # o_proj weight prefetch (flows during attention head 0)
        for g in range(HPC):
            for r8 in range(NCORES):
                nc.sync.dma_start(
                    ow_t[g][r8][:], owP.ap()[ts(HPC * r8 + g, 128), :]
                )

        # ---------------- attention + interleaved o_proj ----------------
        ctx3 = ExitStack()
        probp = ctx3.enter_context(tc.tile_pool(name="probp", bufs=3))
        attup = ctx3.enter_context(tc.tile_pool(name="attup", bufs=4))
        attp = ctx3.enter_context(tc.tile_pool(name="attp", bufs=4))
        denp = ctx3.enter_context(tc.tile_pool(name="denp", bufs=2))
        g2hp = ctx3.enter_context(tc.tile_pool(name="g2hp", bufs=2))
        sc_ps = ctx3.enter_context(tc.tile_pool(name="sc_ps", bufs=2, space="PSUM"))
        av_ps = ctx3.enter_context(tc.tile_pool(name="av_ps", bufs=2, space="PSUM"))
        den_ps = ctx3.enter_context(tc.tile_pool(name="den_ps", bufs=1, space="PSUM"))
        bc_ps = ctx3.enter_context(tc.tile_pool(name="bc_ps", bufs=1, space="PSUM"))
        o_ps = ctx3.enter_context(tc.tile_pool(name="o_ps", bufs=2, space="PSUM"))

        def attention_head(h, flush_prev):
            """Emit head h. Returns a flush closure that emits the
            normalization + g2 writes + AllGather trigger for this head;
            the caller schedules it into a later phase so the PE never
            waits on the softmax reciprocals."""
            kn = kn_all[h]
            pe_rhs = qfpe[h // 2][ds(64 * (h % 2), 64), :]
            pe_lhs = kpe0 if h % 2 == 0 else kx[ds(64, 64), :]
            den_all = denp.tile([128, 512], F, tag="den_all", name=f"denall{h}")
            rinv_all = denp.tile([128, 512], R, tag="rinv_all", name=f"rinvall{h}")
            # unwritten rows are read (x0) by the selector matmul: keep finite
            nc.vector.memset(den_all[:], 1.0)
            attu = []
            for qb in range(4):
                nkt = 4 * (qb + 1)
                avt = av_ps.tile([128, 512], F, tag="av", name=f"av{h}_{qb}")
                dent = den_ps.tile([1, 512], F, tag="den", name=f"den{h}_{qb}")
                sct = [None] * nkt
                prob = [None] * nkt

                def emit_score(kt):
                    trim = max(0, 128 * (kt - 4 * qb))
                    qsl = ds(512 * qb + trim, 512 - trim)
                    sct[kt] = sc_ps.tile([128, 512], F, tag="sc", name=f"sc{h}{qb}_{kt}")
                    nc.tensor.matmul(
                        sct[kt][:, trim:512], kn[:, ts(kt, 128)], qnope[h][:, qsl],
                        start=True, stop=False,
                    )
                    nc.tensor.matmul(
                        sct[kt][:, trim:512], pe_lhs[:, ts(kt, 128)], pe_rhs[:, qsl],
                        start=False, stop=(kt < 4 * qb),
                    )
                    if kt >= 4 * qb:
                        nc.tensor.matmul(
                            sct[kt][:, trim : trim + 128], maskT[:], identh[:],
                            start=False, stop=True,
                        )
                    prob[kt] = probp.tile(
                        [128, 512], F16, tag="prob", name=f"pr{h}{qb}_{kt}"
                    )
                    nc.scalar.activation(
                        prob[kt][:, trim:512], sct[kt][:, trim:512], AF.Exp
                    )

                def emit_avden(kt):
                    trim = max(0, 128 * (kt - 4 * qb))
                    nc.tensor.matmul(
                        avt[:, trim:512], v_sb[:, kt, ts(h, 128)],
                        prob[kt][:, trim:512],
                        start=(kt == 0), stop=(kt == nkt - 1),
                    )
                    nc.tensor.matmul(
                        dent[:, trim:512], ones_col[:], prob[kt][:, trim:512],
                        start=(kt == 0), stop=(kt == nkt - 1),
                    )

                for kt in range(nkt):
                    emit_score(kt)
                    if qb == 1 and kt == 1 and flush_prev is not None:
                        flush_prev()
                        flush_prev = None
                    if kt > 0:
                        emit_avden(kt - 1)
                emit_avden(nkt - 1)
                nc.vector.tensor_copy(den_all[ds(32 * qb, 1), :], dent[:])
                at = attup.tile([128, 512], F, tag="attu", name=f"attu{h}_{qb}")
                nc.scalar.copy(at[:], avt[:])
                attu.append(at)
            with nc.allow_low_precision(reason="f32r out is fp32 storage"):
                nc.vector.reciprocal(rinv_all[:], den_all[:])
            if flush_prev is not None:
                flush_prev()

            def flush():
                for qb in range(4):
                    bc = bc_ps.tile([128, 512], F, tag="bc", name=f"bc{h}_{qb}")
                    nc.tensor.matmul(
                        bc[:], sel4[qb][:], rinv_all[:], start=True, stop=True,
                    )
                    att = attp.tile([128, 512], F16, tag="att", name=f"att{h}_{qb}")
                    nc.vector.tensor_tensor(att[:], attu[qb][:], bc[:], ALU.mult)
                    nc.gpsimd.dma_start(g2_src[h].ap()[:, ts(qb, 512)], att[:])
                nc.gpsimd.collective_compute(
                    "AllGather", ALU.bypass,
                    ins=[g2_src[h].ap().opt()], outs=[g2[h].ap().opt()],
                    replica_groups=RG,
                )

            return flush

        def oproj_group(g, flush_prev=None):
            for sg in range(4):  # 512-col seq block
                g2h = g2hp.tile([128, NCORES, 512], F16, tag="g2h", name=f"g2h{g}_{sg}")
                nc.sync.dma_start(
                    g2h[:],
                    g2[g].ap()[:, :, ts(sg, 512)].rearrange("c p s -> p c s"),
                )
                for mt in range(4):
                    ps = o_ps.tile([128, 512], F, tag="o", name=f"ops{g}_{sg}_{mt}")
                    for r8 in range(NCORES):
                        nc.tensor.matmul(
                            ps[:], ow_t[g][r8][:, ts(mt, 128)],
                            g2h[:, r8, :],
                            start=(r8 == 0), stop=(r8 == NCORES - 1),
                        )
                    if g == 0:
                        nc.scalar.copy(osb[mt][:, sg, :], ps[:])
                    else:
                        nc.vector.tensor_tensor(
                            osb[mt][:, sg, :], osb[mt][:, sg, :], ps[:], ALU.add
                        )
                    if flush_prev is not None and sg == 0 and mt == 1:
                        flush_prev()
                        flush_prev = None

        f0 = attention_head(0, None)
        f1 = attention_head(1, f0)
        f2 = attention_head(2, f1)
        oproj_group(0, flush_prev=f2)
        f3 = attention_head(3, None)
        oproj_group(1, flush_prev=f3)
        oproj_group(2)
        oproj_group(3)

        for mt in range(4):
            nc.gpsimd.dma_start(
                out_d.ap()[ts(mt, 128), :],
                osb[mt][:].rearrange("p sg c -> p (sg c)"),
            )
        ctx3.close()

    nc.compile()
    return nc


def _get_nc():
    global _CACHED_NC
    if _CACHED_NC is None:
        _CACHED_NC = _build_program()
    return _CACHED_NC


def _prep_inputs(hidden_states, position_ids, q_a_w, q_a_ln_w, q_b_w, kv_a_w,
                 kv_a_ln_w, kv_b_w, o_w):
    hidden_states = np.asarray(hidden_states, dtype=np.float32)
    position_ids = np.asarray(position_ids, dtype=np.int32)
    q_a_w = np.asarray(q_a_w, dtype=np.float32)
    q_a_ln_w = np.asarray(q_a_ln_w, dtype=np.float32)
    q_b_w = np.asarray(q_b_w, dtype=np.float32)
    kv_a_w = np.asarray(kv_a_w, dtype=np.float32)
    kv_a_ln_w = np.asarray(kv_a_ln_w, dtype=np.float32)
    kv_b_w = np.asarray(kv_b_w, dtype=np.float32)
    o_w = np.asarray(o_w, dtype=np.float32)

    xT_full = hidden_states[0].T                                 # (H, S)
    # weight tiles packed in exact kernel load order (contiguous DMA)
    qa = q_a_w.T.reshape(4, 8, 128, 3, 512)                      # (ktg,kk,p,ch,n)
    qawP = np.ascontiguousarray(
        qa.transpose(3, 0, 2, 1, 4).reshape(12, 128, 8 * 512)
    ).astype(np.float16)
    kva = kv_a_w.T.reshape(4, 8, 128, 576)                       # (ktg,kk,p,m)
    kvawP = np.ascontiguousarray(
        kva.transpose(0, 2, 1, 3).reshape(4, 128, 8 * 576)
    ).astype(np.float16)

    identH = np.eye(128, dtype=np.float16)
    maskT = np.zeros((128, 128), dtype=np.float32)
    maskT[np.triu_indices(128, 1)] = MASKV                       # cols > rows
    maskT = maskT.astype(np.float16)
    selP = np.zeros((4, 128, 128), dtype=np.float32)
    for qb in range(4):
        selP[qb, 32 * qb, :] = 1.0

    # rope tables from position_ids (host-computed)
    inv_freq = (1.0 / (THETA ** (np.arange(0, DR, 2) / DR))).astype(np.float64)
    pos = position_ids.reshape(-1).astype(np.float64)            # (S,)
    ang_q = pos[None, :] * np.tile(inv_freq, 4)[:, None]         # (128, S)
    cosqP = np.cos(ang_q).astype(np.float32)
    sinqP = np.sin(ang_q).astype(np.float32)

    owT_full = o_w.T                                             # (N*DV, H)

    in_maps = []
    for c in range(NCORES):
        heads = slice(HPC * c, HPC * (c + 1))
        qb = q_b_w.reshape(N_HEADS, QD, QR)[heads]               # (4, 192, QR)
        nope = qb[:, :DN, :].reshape(HPC * DN, QR)
        pe = qb[:, DN:, :]
        pe_d = np.concatenate([pe[:, 0::2, :], pe[:, 1::2, :]], axis=1)  # (4,64,QR)
        pe_r = np.concatenate([-pe_d[:, 32:, :], pe_d[:, :32, :]], axis=1)
        cols = np.concatenate(
            [nope, pe_d.reshape(HPC * DR, QR), pe_r.reshape(HPC * DR, QR)], axis=0
        )                                                        # (1024, QR)
        qbwT_c = (cols * (SCALE * q_a_ln_w[None, :])).T          # (QR, 1024)
        qbwP = np.ascontiguousarray(qbwT_c.reshape(NR, 128, 1024)).astype(np.float16)

        kvb = kv_b_w.reshape(N_HEADS, DN + DV, KR)[heads]
        kcols = np.concatenate(
            [kvb[:, :DN, :].reshape(HPC * DN, KR),
             kvb[:, DN:, :].reshape(HPC * DV, KR)],
            axis=0,
        )                                                        # (1024, KR)
        kvbwT_c = (kcols * kv_a_ln_w[None, :]).T                 # (KR, 1024)
        kvbwP = np.ascontiguousarray(kvbwT_c.reshape(NKR, 128, 1024)).astype(np.float16)

        # x tile: xP[p, kt*SL + s] = x[kt*128+p, SL*c + s]
        xs = xT_full[:, SL * c : SL * (c + 1)].reshape(NKT, 128, SL)
        xP = np.ascontiguousarray(xs.transpose(1, 0, 2).reshape(128, NKT * SL)
                                  ).astype(np.float16)

        # k-rope tables for this core's 256 tokens: [st, p, j]
        posc = pos[SL * c : SL * (c + 1)].reshape(2, 128)        # (st, p)
        ang_k = posc[:, :, None] * inv_freq[None, None, :]       # (2, 128, 32)
        coskP = np.cos(ang_k).astype(np.float32)
        sinkP = np.sin(ang_k).astype(np.float32)

        in_maps.append(
            {
                "xP": xP,
                "qawP": qawP,
                "kvawP": kvawP,
                "qbwP": qbwP,
                "kvbwP": kvbwP,
                "owP": np.ascontiguousarray(
                    owT_full[:, MCOLS * c : MCOLS * (c + 1)]
                ).astype(np.float16),
                "cosqP": cosqP,
                "sinqP": sinqP,
                "coskP": coskP,
                "sinkP": sinkP,
                "identH": identH,
                "maskTP": maskT,
                "selP": selP,
            }
        )
    return in_maps


def kernel(**inputs):
    global LAST_RESULT
    nc = _get_nc()
    in_maps = _prep_inputs(**inputs)
    res = run_bass_kernel_spmd(nc, in_maps, list(range(NCORES)))
    LAST_RESULT = res
    out = np.concatenate([res.results[c]["out"].T for c in range(NCORES)], axis=1)
    return out[None].astype(np.float32)


# revision 58
# speedup vs baseline: 1.0134x; 1.0134x over previous
"""DeepSeekV3 MLA attention prefill kernel for 8 Trainium2 NeuronCores.

Sharding: sequence-parallel for the low-rank input projections (q_a / kv_a),
AllGather of the shared latents, tensor-parallel over heads (4 heads/core) for
q_b / kv_b decompression and attention, AllGather of attention outputs, and
output-column-parallel o_proj (each core produces a disjoint 512-column slice
of the output, concatenated on host).

v3: float16 operands, host-packed contiguous weight tiles, host-precomputed
rope tables, causal mask folded into the score matmul, XBAR DMA-transpose for
the latent gather (no PE transposes), decompress-input loads on the Act DMA
queue so decompression starts the moment the PE frees up, per-q-block softmax
reciprocals with head-normalization deferred into the next phase, and o_proj
groups scheduled h0 h1 h2 g0 h3 g1 g2 g3 so every AllGather hides.
"""

import sys

sys.path.insert(0, "/opt/trn_rl_repo")

import numpy as np

import concourse.bass as bass  # noqa: F401
import concourse.mybir as mybir
from concourse import bacc
from concourse.bass import ds, ts
from concourse.tile import TileContext
from concourse.bass_utils import run_bass_kernel_spmd
from contextlib import ExitStack

F = mybir.dt.float32
F16 = mybir.dt.float16
R = mybir.dt.float32r
AF = mybir.ActivationFunctionType
ALU = mybir.AluOpType

NCORES = 8
B, S, H = 1, 2048, 4096
N_HEADS = 32
HPC = N_HEADS // NCORES          # heads per core = 4
SL = S // NCORES                 # sequence rows per core = 256
QR, KR = 1536, 512
DR, DN, DV = 64, 128, 128
QD = DN + DR                     # 192
SCALE = QD ** -0.5
EPS = 1e-6
THETA = 10000.0
MCOLS = H // NCORES              # output columns per core = 512
NKT = H // 128                   # 32 k-tiles over the model dim
NR = QR // 128                   # 12 k-tiles over q_lora_rank
NKR = KR // 128                  # 4 k-tiles over kv_lora_rank
KD = KR + DR                     # 576
MASKV = -30000.0

LAST_RESULT = None               # test harness reads exec_time_ns from here
_CACHED_NC = None


def _build_program():
    nc = bacc.Bacc(None, target_bir_lowering=False, num_devices=NCORES)

    # ---------------- DRAM declarations ----------------
    xP = nc.dram_tensor("xP", [128, NKT * SL], F16, kind="ExternalInput")
    qawP = nc.dram_tensor("qawP", [12, 128, 8 * 512], F16, kind="ExternalInput")
    kvawP = nc.dram_tensor("kvawP", [4, 128, 8 * 576], F16, kind="ExternalInput")
    qbwP = nc.dram_tensor("qbwP", [NR, 128, 1024], F16, kind="ExternalInput")
    kvbwP = nc.dram_tensor("kvbwP", [NKR, 128, 1024], F16, kind="ExternalInput")
    owP = nc.dram_tensor("owP", [N_HEADS * DV, MCOLS], F16, kind="ExternalInput")
    cosqP = nc.dram_tensor("cosqP", [128, S], F, kind="ExternalInput")
    sinqP = nc.dram_tensor("sinqP", [128, S], F, kind="ExternalInput")
    coskP = nc.dram_tensor("coskP", [2, 128, 32], F, kind="ExternalInput")
    sinkP = nc.dram_tensor("sinkP", [2, 128, 32], F, kind="ExternalInput")
    identH = nc.dram_tensor("identH", [128, 128], F16, kind="ExternalInput")
    maskTP = nc.dram_tensor("maskTP", [128, 128], F16, kind="ExternalInput")
    selP = nc.dram_tensor("selP", [4, 128, 128], F, kind="ExternalInput")
    out_d = nc.dram_tensor("out", [MCOLS, S], F, kind="ExternalOutput")

    # latents gathered in [token, feature] layout; consumers transpose-read
    g1kv_src = nc.dram_tensor("g1kv_src", [SL, KD], F16)
    g1kv = nc.dram_tensor("g1kv", [NCORES, SL, KD], F16, addr_space="Shared")
    g1q_src = nc.dram_tensor("g1q_src", [SL, QR], F16)
    g1q = nc.dram_tensor("g1q", [NCORES, SL, QR], F16, addr_space="Shared")
    g2_src = [nc.dram_tensor(f"g2src{p}", [2, DV, S], F16) for p in range(2)]
    g2 = [
        nc.dram_tensor(f"g2_{p}", [NCORES, 2, DV, S], F16, addr_space="Shared")
        for p in range(2)
    ]
    RG = [list(range(NCORES))]
    g1kv_flat = g1kv.ap().rearrange("c s f -> (c s) f")
    g1q_flat = g1q.ap().rearrange("c s f -> (c s) f")

    with TileContext(nc) as tc, ExitStack() as ctx:
        persist = ctx.enter_context(tc.tile_pool(name="persist", bufs=1))
        ckvp = ctx.enter_context(tc.tile_pool(name="ckvp", bufs=1))



        # input activations + first weight tiles first — the first matmul
        # waits only on chunk 0; constants follow (not needed until rms)
        xtp = ExitStack()
        xtpool = xtp.enter_context(tc.tile_pool(name="xtp", bufs=1))
        xt = xtpool.tile([128, NKT, SL], F16, name="xt")
        for g in range(4):
            nc.sync.dma_start(
                xt[:, ds(8 * g, 8), :],
                xP.ap()[:, ds(8 * g * SL, 8 * SL)].rearrange(
                    "p (kt s) -> p kt s", kt=8
                ),
            )
        wp0_e = ExitStack()
        wp0 = wp0_e.enter_context(tc.tile_pool(name="wp0", bufs=4))
        kvw_tiles = []
        for ktg in range(4):
            w = wp0.tile([128, 8, 576], F16, tag="kvw", name=f"kvw{ktg}")
            nc.sync.dma_start(
                w[:], kvawP.ap()[ktg].rearrange("p (k n) -> p k n", k=8)
            )
            kvw_tiles.append(w)

        # ---------------- constants ----------------
        identh = persist.tile([128, 128], F16, name="c_identh")
        nc.sync.dma_start(identh[:], identH[:])
        maskT = persist.tile([128, 128], F16, name="c_maskT")
        nc.sync.dma_start(maskT[:], maskTP[:])
        ones_col = persist.tile([128, 1], F16, name="c_ones_col")
        nc.vector.memset(ones_col[:], 1.0)
        sel4 = []
        for qb in range(4):
            s = persist.tile([128, 128], R, name=f"c_sel{qb}")
            nc.sync.dma_start(s[:], selP.ap()[qb].bitcast(R))
            sel4.append(s)
        eps_t = persist.tile([128, 1], F, name="c_eps")
        nc.vector.memset(eps_t[:], EPS)
        sin_k = [persist.tile([128, 32], F, name=f"t_sink{st}") for st in range(2)]
        cos_k = [persist.tile([128, 32], F, name=f"t_cosk{st}") for st in range(2)]
        for st in range(2):
            nc.sync.dma_start(sin_k[st][:], sinkP.ap()[st])
            nc.sync.dma_start(cos_k[st][:], coskP.ap()[st])

        # persistent attention-phase state
        kx = persist.tile([128, S], F16, name="kx")        # [64:128] = k_pe
        kpe0 = persist.tile([64, S], F16, name="kpe0")     # k_pe for even heads
        qnope = [persist.tile([128, S], F16, name=f"qnope{h}") for h in range(HPC)]
        qfpe = [persist.tile([128, S], F16, name=f"qfpe{p}") for p in range(2)]
        kn_all = [persist.tile([128, S], F16, name=f"kn{h}") for h in range(HPC)]
        v_sb = persist.tile([128, S // 128, 512], F16, name="v_sb")

        # ---------------- phase 0: q_a / kv_a projections ----------------
        ctx0 = ExitStack()
        p0 = ctx0.enter_context(tc.tile_pool(name="p0", bufs=2))

        # --- kv_a: 512-col ckv chunk + 64-col k_pe chunk per st ---
        ctx0a = ExitStack()
        kv0_ps = ctx0a.enter_context(tc.tile_pool(name="kv0_ps", bufs=2, space="PSUM"))
        kv1_ps = ctx0a.enter_context(tc.tile_pool(name="kv1_ps", bufs=2, space="PSUM"))

        kvch = [
            kv0_ps.tile([128, 512], F, tag="kv0", name=f"kvps{st}") for st in range(2)
        ]
        kvpe = [
            kv1_ps.tile([128, 64], F, tag="kv1", name=f"kvpe{st}") for st in range(2)
        ]
        for ktg in range(4):
            w = kvw_tiles[ktg]
            for kk in range(8):
                kt = ktg * 8 + kk
                for st in range(2):
                    nc.tensor.matmul(
                        kvch[st][:], xt[:, kt, ts(st, 128)], w[:, kk, 0:512],
                        start=(kt == 0), stop=(kt == NKT - 1),
                    )
                    nc.tensor.matmul(
                        kvpe[st][:], xt[:, kt, ts(st, 128)], w[:, kk, 512:576],
                        start=(kt == 0), stop=(kt == NKT - 1),
                    )
        for st in range(2):
            acc0 = p0.tile([128, 1], F, tag="kvacc", name=f"kvacc{st}")
            scr = p0.tile([128, 512], F, tag="kvscr", name=f"kvscr{st}")
            nc.scalar.activation(scr[:], kvch[st][:], AF.Square, accum_out=acc0[:])
            stdv = p0.tile([128, 1], F, tag="kvstd", name=f"kvstd{st}")
            nc.scalar.activation(
                stdv[:], acc0[:], AF.Sqrt, bias=eps_t[:], scale=1.0 / KR
            )
            rinv = p0.tile([128, 1], F, tag="kvrinv", name=f"kvrinv{st}")
            nc.vector.reciprocal(rinv[:], stdv[:])
            ckvn = p0.tile([128, KR], F16, tag="ckvn", name=f"ckvn{st}")
            nc.vector.tensor_scalar_mul(ckvn[:], kvch[st][:], rinv[:])
            nc.gpsimd.dma_start(g1kv_src.ap()[ts(st, 128), 0:KR], ckvn[:])
            # rope k_pe: deinterleave even/odd from the 64-col chunk
            pe = kvpe[st][:].rearrange("p (d two) -> p two d", two=2)
            y1, y2 = pe[:, 0], pe[:, 1]
            kr_t = p0.tile([128, DR], F16, tag="kr", name=f"kr{st}")
            t1 = p0.tile([128, 32], F, tag="krt1", name=f"krt1_{st}")
            t2 = p0.tile([128, 32], F, tag="krt2", name=f"krt2_{st}")
            nc.vector.tensor_tensor(t1[:], y1, cos_k[st][:], ALU.mult)
            nc.vector.tensor_tensor(t2[:], y2, sin_k[st][:], ALU.mult)
            nc.vector.tensor_tensor(kr_t[:, 0:32], t1[:], t2[:], ALU.subtract)
            nc.vector.tensor_tensor(t1[:], y2, cos_k[st][:], ALU.mult)
            nc.vector.tensor_tensor(t2[:], y1, sin_k[st][:], ALU.mult)
            nc.vector.tensor_tensor(kr_t[:, 32:64], t1[:], t2[:], ALU.add)
            nc.gpsimd.dma_start(g1kv_src.ap()[ts(st, 128), KR:KD], kr_t[:])
        ctx0a.close()

        nc.gpsimd.collective_compute(
            "AllGather", ALU.bypass,
            ins=[g1kv_src.ap().opt()], outs=[g1kv.ap().opt()], replica_groups=RG,
        )

        # --- q_a (3 chunks of 512 cols) ---
        ctx0b = ExitStack()
        qa_ps = ctx0b.enter_context(tc.tile_pool(name="qa_ps", bufs=6, space="PSUM"))
        qch = [[None] * 3 for _ in range(2)]
        for ch in range(3):
            for st in range(2):
                qch[st][ch] = qa_ps.tile(
                    [128, 512], F, tag="qa_ps", name=f"qaps{st}_{ch}"
                )
        for ch in range(3):
            for ktg in range(4):
                w = wp0.tile([128, 8, 512], F16, tag="qaw", name=f"qaw{ch}_{ktg}")
                nc.sync.dma_start(
                    w[:], qawP.ap()[ch * 4 + ktg].rearrange("p (k n) -> p k n", k=8)
                )
                for kk in range(8):
                    kt = ktg * 8 + kk
                    for st in range(2):
                        nc.tensor.matmul(
                            qch[st][ch][:], xt[:, kt, ts(st, 128)], w[:, kk],
                            start=(kt == 0), stop=(kt == NKT - 1),
                        )
        for st in range(2):
            accs = []
            scr = p0.tile([128, 512], F, tag="qascr", name=f"qascr{st}")
            for ch in range(3):
                a = p0.tile([128, 1], F, tag="qaacc", name=f"qaacc{st}_{ch}")
                nc.scalar.activation(scr[:], qch[st][ch][:], AF.Square, accum_out=a[:])
                accs.append(a)
            nc.vector.tensor_tensor(accs[0][:], accs[0][:], accs[1][:], ALU.add)
            nc.vector.tensor_tensor(accs[0][:], accs[0][:], accs[2][:], ALU.add)
            stdv = p0.tile([128, 1], F, tag="qastd", name=f"qastd{st}")
            nc.scalar.activation(
                stdv[:], accs[0][:], AF.Sqrt, bias=eps_t[:], scale=1.0 / QR
            )
            rinv = p0.tile([128, 1], F, tag="qarinv", name=f"qarinv{st}")
            nc.vector.reciprocal(rinv[:], stdv[:])
            qn = p0.tile([128, QR], F16, tag="qn", name=f"qn{st}")
            for ch in range(3):
                nc.vector.tensor_scalar_mul(qn[:, ts(ch, 512)], qch[st][ch][:], rinv[:])
            nc.gpsimd.dma_start(g1q_src.ap()[ts(st, 128), :], qn[:])
        ctx0b.close()
        ctx0.close()
        wp0_e.close()
        xtp.close()

        nc.gpsimd.collective_compute(
            "AllGather", ALU.bypass,
            ins=[g1q_src.ap().opt()], outs=[g1q.ap().opt()], replica_groups=RG,
        )

        # decompress inputs (transpose-read of the gathered kv latent)
        kvbw = []
        for r in range(NKR):
            t = ckvp.tile([128, 1024], F16, name=f"kvbw{r}")
            nc.sync.dma_start(t[:], kvbwP.ap()[r])
            kvbw.append(t)
        ckv_t = []
        for r in range(NKR):
            t = ckvp.tile([128, S], F16, name=f"ckv{r}")
            nc.sync.dma_start_transpose(t[:], g1kv_flat[:, ts(r, 128)])
            ckv_t.append(t)
        nc.sync.dma_start_transpose(kx[:], g1kv_flat[:, KD - 128 : KD])
        nc.gpsimd.dma_start(kpe0[:], kx[ds(64, 64), :])

        # o_proj weights / output accumulators (space freed by phase 0)
        persist2 = ctx.enter_context(tc.tile_pool(name="persist2", bufs=1))
        osb = [persist2.tile([128, 4, 512], F, name=f"osb{mt}") for mt in range(4)]
        ow_t = [
            [persist2.tile([128, MCOLS], F16, name=f"ow{g}_{r8}")
             for r8 in range(NCORES)]
            for g in range(HPC)
        ]

        # ---------------- decompression (overlaps the q AllGather) ----------
        dec_ctx = ExitStack()
        dec_ps = dec_ctx.enter_context(tc.tile_pool(name="dec_ps", bufs=4, space="PSUM"))
        for st in range(S // 128):
            ps = dec_ps.tile([128, 512], F, tag="dec", name=f"vps{st}")
            for r in range(NKR):
                nc.tensor.matmul(
                    ps[:], ckv_t[r][:, ts(st, 128)], kvbw[r][:, 512:1024],
                    start=(r == 0), stop=(r == NKR - 1),
                )
            nc.vector.tensor_copy(v_sb[:, st, :], ps[:])
        for h in range(HPC):
            for sb in range(4):
                ps = dec_ps.tile([128, 512], F, tag="dec", name=f"knps{h}_{sb}")
                for r in range(NKR):
                    nc.tensor.matmul(
                        ps[:], kvbw[r][:, ts(h, 128)], ckv_t[r][:, ts(sb, 512)],
                        start=(r == 0), stop=(r == NKR - 1),
                    )
                nc.scalar.copy(kn_all[h][:, ts(sb, 512)], ps[:])
        dec_ctx.close()

        # ---------------- q_b projection (+ q rope) ----------------
        ctx2 = ExitStack()
        qbwp = ctx2.enter_context(tc.tile_pool(name="qbwp", bufs=1))
        qrp = ctx2.enter_context(tc.tile_pool(name="qrp", bufs=12))
        ropep = ctx2.enter_context(tc.tile_pool(name="ropep", bufs=1))
        sinp = ctx2.enter_context(tc.tile_pool(name="sinp", bufs=2))
        qb_ps = ctx2.enter_context(tc.tile_pool(name="qb_ps", bufs=8, space="PSUM"))

        qbw = []
        for r in range(NR):
            t = qbwp.tile([128, 1024], F16, name=f"qbw{r}")
            nc.sync.dma_start(t[:], qbwP.ap()[r])
            qbw.append(t)

        for sb in range(4):
            sin_q = sinp.tile([128, 512], F, tag="sinq", name=f"t_sinq{sb}")
            cos_q = sinp.tile([128, 512], F, tag="cosq", name=f"t_cosq{sb}")
            nc.sync.dma_start(sin_q[:], sinqP.ap()[:, ts(sb, 512)])
            nc.sync.dma_start(cos_q[:], cosqP.ap()[:, ts(sb, 512)])
            qr_tiles = []
            for r in range(NR):
                t = qrp.tile([128, 512], F16, tag="qr", name=f"qr{sb}_{r}")
                nc.sync.dma_start_transpose(
                    t[:], g1q_flat[ds(2 * sb * SL, 2 * SL), ts(r, 128)]
                )
                qr_tiles.append(t)
            rope_t1 = {}
            for j in range(8):
                psj = qb_ps.tile([128, 512], F, tag="qb_ps", name=f"qbps{sb}_{j}")
                for r in range(NR):
                    nc.tensor.matmul(
                        psj[:], qbw[r][:, ts(j, 128)], qr_tiles[r][:],
                        start=(r == 0), stop=(r == NR - 1),
                    )
                if j < 4:
                    nc.scalar.copy(qnope[j][:, ts(sb, 512)], psj[:])
                elif j < 6:
                    p = j - 4
                    t1 = ropep.tile([128, 512], F, tag=f"rope1_{p}", name=f"rp1_{sb}_{p}")
                    nc.vector.tensor_tensor(t1[:], psj[:], cos_q[:], ALU.mult)
                    rope_t1[p] = t1
                else:
                    p = j - 6
                    t2 = ropep.tile([128, 512], F, tag=f"rope2_{p}", name=f"rp2_{sb}_{p}")
                    nc.vector.tensor_tensor(t2[:], psj[:], sin_q[:], ALU.mult)
                    nc.vector.tensor_tensor(
                        qfpe[p][:, ts(sb, 512)], rope_t1[p][:], t2[:], ALU.add
                    )
        ctx2.close()

        # o_proj weight prefetch (flows during attention head 0)
        for g in range(HPC):
            for r8 in range(NCORES):
                nc.sync.dma_start(
                    ow_t[g][r8][:], owP.ap()[ts(HPC * r8 + g, 128), :]
                )

        # ---------------- attention + interleaved o_proj ----------------
        ctx3 = ExitStack()
        probp = ctx3.enter_context(tc.tile_pool(name="probp", bufs=3))
        attup = ctx3.enter_context(tc.tile_pool(name="attup", bufs=4))
        attp = ctx3.enter_context(tc.tile_pool(name="attp", bufs=4))
        denp = ctx3.enter_context(tc.tile_pool(name="denp", bufs=2))
        g2hp = ctx3.enter_context(tc.tile_pool(name="g2hp", bufs=2))
        sc_ps = ctx3.enter_context(tc.tile_pool(name="sc_ps", bufs=2, space="PSUM"))
        av_ps = ctx3.enter_context(tc.tile_pool(name="av_ps", bufs=2, space="PSUM"))
        den_ps = ctx3.enter_context(tc.tile_pool(name="den_ps", bufs=1, space="PSUM"))
        bc_ps = ctx3.enter_context(tc.tile_pool(name="bc_ps", bufs=1, space="PSUM"))
        o_ps = ctx3.enter_context(tc.tile_pool(name="o_ps", bufs=2, space="PSUM"))

        def attention_head(h, flush_prev):
            """Emit head h. Returns a flush closure that emits the
            normalization + g2 writes + AllGather trigger for this head;
            the caller schedules it into a later phase so the PE never
            waits on the softmax reciprocals."""
            kn = kn_all[h]
            pe_rhs = qfpe[h // 2][ds(64 * (h % 2), 64), :]
            pe_lhs = kpe0 if h % 2 == 0 else kx[ds(64, 64), :]
            den_all = denp.tile([128, 512], F, tag="den_all", name=f"denall{h}")
            rinv_all = denp.tile([128, 512], R, tag="rinv_all", name=f"rinvall{h}")
            # unwritten rows are read (x0) by the selector matmul: keep finite
            nc.vector.memset(den_all[:], 1.0)
            attu = []
            for qb in range(4):
                nkt = 4 * (qb + 1)
                avt = av_ps.tile([128, 512], F, tag="av", name=f"av{h}_{qb}")
                dent = den_ps.tile([1, 512], F, tag="den", name=f"den{h}_{qb}")
                sct = [None] * nkt
                prob = [None] * nkt

                def emit_score(kt):
                    trim = max(0, 128 * (kt - 4 * qb))
                    qsl = ds(512 * qb + trim, 512 - trim)
                    sct[kt] = sc_ps.tile([128, 512], F, tag="sc", name=f"sc{h}{qb}_{kt}")
                    nc.tensor.matmul(
                        sct[kt][:, trim:512], kn[:, ts(kt, 128)], qnope[h][:, qsl],
                        start=True, stop=False,
                    )
                    nc.tensor.matmul(
                        sct[kt][:, trim:512], pe_lhs[:, ts(kt, 128)], pe_rhs[:, qsl],
                        start=False, stop=(kt < 4 * qb),
                    )
                    if kt >= 4 * qb:
                        nc.tensor.matmul(
                            sct[kt][:, trim : trim + 128], maskT[:], identh[:],
                            start=False, stop=True,
                        )
                    prob[kt] = probp.tile(
                        [128, 512], F16, tag="prob", name=f"pr{h}{qb}_{kt}"
                    )
                    nc.scalar.activation(
                        prob[kt][:, trim:512], sct[kt][:, trim:512], AF.Exp
                    )

                def emit_avden(kt):
                    trim = max(0, 128 * (kt - 4 * qb))
                    nc.tensor.matmul(
                        avt[:, trim:512], v_sb[:, kt, ts(h, 128)],
                        prob[kt][:, trim:512],
                        start=(kt == 0), stop=(kt == nkt - 1),
                    )
                    nc.tensor.matmul(
                        dent[:, trim:512], ones_col[:], prob[kt][:, trim:512],
                        start=(kt == 0), stop=(kt == nkt - 1),
                    )

                for kt in range(nkt):
                    emit_score(kt)
                    if qb == 1 and kt == 1 and flush_prev is not None:
                        flush_prev()
                        flush_prev = None
                    if kt > 0:
                        emit_avden(kt - 1)
                emit_avden(nkt - 1)
                nc.vector.tensor_copy(den_all[ds(32 * qb, 1), :], dent[:])
                at = attup.tile([128, 512], F, tag="attu", name=f"attu{h}_{qb}")
                nc.scalar.copy(at[:], avt[:])
                attu.append(at)
            with nc.allow_low_precision(reason="f32r out is fp32 storage"):
                nc.vector.reciprocal(rinv_all[:], den_all[:])
            if flush_prev is not None:
                flush_prev()

            def flush():
                for qb in range(4):
                    bc = bc_ps.tile([128, 512], F, tag="bc", name=f"bc{h}_{qb}")
                    nc.tensor.matmul(
                        bc[:], sel4[qb][:], rinv_all[:], start=True, stop=True,
                    )
                    att = attp.tile([128, 512], F16, tag="att", name=f"att{h}_{qb}")
                    nc.vector.tensor_tensor(att[:], attu[qb][:], bc[:], ALU.mult)
                    nc.gpsimd.dma_start(
                        g2_src[h // 2].ap()[h % 2, :, ts(qb, 512)], att[:]
                    )
                if h % 2 == 1:
                    p = h // 2
                    nc.gpsimd.collective_compute(
                        "AllGather", ALU.bypass,
                        ins=[g2_src[p].ap().opt()], outs=[g2[p].ap().opt()],
                        replica_groups=RG,
                    )

            return flush

        def oproj_group(g, flush_prev=None):
            for sg in range(4):  # 512-col seq block
                g2h = g2hp.tile([128, NCORES, 512], F16, tag="g2h", name=f"g2h{g}_{sg}")
                nc.sync.dma_start(
                    g2h[:],
                    g2[g // 2].ap()[:, g % 2, :, ts(sg, 512)]
                    .rearrange("c p s -> p c s"),
                )
                for mt in range(4):
                    ps = o_ps.tile([128, 512], F, tag="o", name=f"ops{g}_{sg}_{mt}")
                    for r8 in range(NCORES):
                        nc.tensor.matmul(
                            ps[:], ow_t[g][r8][:, ts(mt, 128)],
                            g2h[:, r8, :],
                            start=(r8 == 0), stop=(r8 == NCORES - 1),
                        )
                    if g == 0:
                        nc.scalar.copy(osb[mt][:, sg, :], ps[:])
                    else:
                        nc.vector.tensor_tensor(
                            osb[mt][:, sg, :], osb[mt][:, sg, :], ps[:], ALU.add
                        )
                    if flush_prev is not None and sg == 0 and mt == 1:
                        flush_prev()
                        flush_prev = None

        f0 = attention_head(0, None)
        f1 = attention_head(1, f0)
        f2 = attention_head(2, f1)
        f3 = attention_head(3, f2)
        oproj_group(0, flush_prev=f3)
        oproj_group(1)
        oproj_group(2)
        oproj_group(3)

        for mt in range(4):
            nc.gpsimd.dma_start(
                out_d.ap()[ts(mt, 128), :],
                osb[mt][:].rearrange("p sg c -> p (sg c)"),
            )
        ctx3.close()

    nc.compile()
    return nc


def _get_nc():
    global _CACHED_NC
    if _CACHED_NC is None:
        _CACHED_NC = _build_program()
    return _CACHED_NC


def _prep_inputs(hidden_states, position_ids, q_a_w, q_a_ln_w, q_b_w, kv_a_w,
                 kv_a_ln_w, kv_b_w, o_w):
    hidden_states = np.asarray(hidden_states, dtype=np.float32)
    position_ids = np.asarray(position_ids, dtype=np.int32)
    q_a_w = np.asarray(q_a_w, dtype=np.float32)
    q_a_ln_w = np.asarray(q_a_ln_w, dtype=np.float32)
    q_b_w = np.asarray(q_b_w, dtype=np.float32)
    kv_a_w = np.asarray(kv_a_w, dtype=np.float32)
    kv_a_ln_w = np.asarray(kv_a_ln_w, dtype=np.float32)
    kv_b_w = np.asarray(kv_b_w, dtype=np.float32)
    o_w = np.asarray(o_w, dtype=np.float32)

    xT_full = hidden_states[0].T                                 # (H, S)
    # weight tiles packed in exact kernel load order (contiguous DMA)
    qa = q_a_w.T.reshape(4, 8, 128, 3, 512)                      # (ktg,kk,p,ch,n)
    qawP = np.ascontiguousarray(
        qa.transpose(3, 0, 2, 1, 4).reshape(12, 128, 8 * 512)
    ).astype(np.float16)
    kva = kv_a_w.T.reshape(4, 8, 128, 576)                       # (ktg,kk,p,m)
    kvawP = np.ascontiguousarray(
        kva.transpose(0, 2, 1, 3).reshape(4, 128, 8 * 576)
    ).astype(np.float16)

    identH = np.eye(128, dtype=np.float16)
    maskT = np.zeros((128, 128), dtype=np.float32)
    maskT[np.triu_indices(128, 1)] = MASKV                       # cols > rows
    maskT = maskT.astype(np.float16)
    selP = np.zeros((4, 128, 128), dtype=np.float32)
    for qb in range(4):
        selP[qb, 32 * qb, :] = 1.0

    # rope tables from position_ids (host-computed)
    inv_freq = (1.0 / (THETA ** (np.arange(0, DR, 2) / DR))).astype(np.float64)
    pos = position_ids.reshape(-1).astype(np.float64)            # (S,)
    ang_q = pos[None, :] * np.tile(inv_freq, 4)[:, None]         # (128, S)
    cosqP = np.cos(ang_q).astype(np.float32)
    sinqP = np.sin(ang_q).astype(np.float32)

    owT_full = o_w.T                                             # (N*DV, H)

    in_maps = []
    for c in range(NCORES):
        heads = slice(HPC * c, HPC * (c + 1))
        qb = q_b_w.reshape(N_HEADS, QD, QR)[heads]               # (4, 192, QR)
        nope = qb[:, :DN, :].reshape(HPC * DN, QR)
        pe = qb[:, DN:, :]
        pe_d = np.concatenate([pe[:, 0::2, :], pe[:, 1::2, :]], axis=1)  # (4,64,QR)
        pe_r = np.concatenate([-pe_d[:, 32:, :], pe_d[:, :32, :]], axis=1)
        cols = np.concatenate(
            [nope, pe_d.reshape(HPC * DR, QR), pe_r.reshape(HPC * DR, QR)], axis=0
        )                                                        # (1024, QR)
        qbwT_c = (cols * (SCALE * q_a_ln_w[None, :])).T          # (QR, 1024)
        qbwP = np.ascontiguousarray(qbwT_c.reshape(NR, 128, 1024)).astype(np.float16)

        kvb = kv_b_w.reshape(N_HEADS, DN + DV, KR)[heads]
        kcols = np.concatenate(
            [kvb[:, :DN, :].reshape(HPC * DN, KR),
             kvb[:, DN:, :].reshape(HPC * DV, KR)],
            axis=0,
        )                                                        # (1024, KR)
        kvbwT_c = (kcols * kv_a_ln_w[None, :]).T                 # (KR, 1024)
        kvbwP = np.ascontiguousarray(kvbwT_c.reshape(NKR, 128, 1024)).astype(np.float16)

        # x tile: xP[p, kt*SL + s] = x[kt*128+p, SL*c + s]
        xs = xT_full[:, SL * c : SL * (c + 1)].reshape(NKT, 128, SL)
        xP = np.ascontiguousarray(xs.transpose(1, 0, 2).reshape(128, NKT * SL)
                                  ).astype(np.float16)

        # k-rope tables for this core's 256 tokens: [st, p, j]
        posc = pos[SL * c : SL * (c + 1)].reshape(2, 128)        # (st, p)
        ang_k = posc[:, :, None] * inv_freq[None, None, :]       # (2, 128, 32)
        coskP = np.cos(ang_k).astype(np.float32)
        sinkP = np.sin(ang_k).astype(np.float32)

        in_maps.append(
            {
                "xP": xP,
                "qawP": qawP,
                "kvawP": kvawP,
                "qbwP": qbwP,
                "kvbwP": kvbwP,
                "owP": np.ascontiguousarray(
                    owT_full[:, MCOLS * c : MCOLS * (c + 1)]
                ).astype(np.float16),
                "cosqP": cosqP,
                "sinqP": sinqP,
                "coskP": coskP,
                "sinkP": sinkP,
                "identH": identH,
                "maskTP": maskT,
                "selP": selP,
            }
        )
    return in_maps


def kernel(**inputs):
    global LAST_RESULT
    nc = _get_nc()
    in_maps = _prep_inputs(**inputs)
    res = run_bass_kernel_spmd(nc, in_maps, list(range(NCORES)))
    LAST_RESULT = res
    out = np.concatenate([res.results[c]["out"].T for c in range(NCORES)], axis=1)
    return out[None].astype(np.float32)


# revision 65
# speedup vs baseline: 1.0642x; 1.0501x over previous
"""DeepSeekV3 MLA attention prefill kernel for 8 Trainium2 NeuronCores.

Sharding: sequence-parallel for the low-rank input projections (q_a / kv_a),
AllGather of the shared latents, tensor-parallel over heads (4 heads/core) for
q_b / kv_b decompression and attention, AllGather of attention outputs, and
output-column-parallel o_proj (each core produces a disjoint 512-column slice
of the output, concatenated on host).

v3: float16 operands, host-packed contiguous weight tiles, host-precomputed
rope tables, causal mask folded into the score matmul, XBAR DMA-transpose for
the latent gather (no PE transposes), decompress-input loads on the Act DMA
queue so decompression starts the moment the PE frees up, per-q-block softmax
reciprocals with head-normalization deferred into the next phase, and o_proj
groups scheduled h0 h1 h2 g0 h3 g1 g2 g3 so every AllGather hides.
"""

import sys

sys.path.insert(0, "/opt/trn_rl_repo")

import numpy as np

import concourse.bass as bass  # noqa: F401
import concourse.mybir as mybir
from concourse import bacc
from concourse.bass import ds, ts
from concourse.tile import TileContext
from concourse.bass_utils import run_bass_kernel_spmd
from contextlib import ExitStack

F = mybir.dt.float32
F16 = mybir.dt.float16
R = mybir.dt.float32r
AF = mybir.ActivationFunctionType
ALU = mybir.AluOpType

NCORES = 8
B, S, H = 1, 2048, 4096
N_HEADS = 32
HPC = N_HEADS // NCORES          # heads per core = 4
SL = S // NCORES                 # sequence rows per core = 256
QR, KR = 1536, 512
DR, DN, DV = 64, 128, 128
QD = DN + DR                     # 192
SCALE = QD ** -0.5
EPS = 1e-6
THETA = 10000.0
MCOLS = H // NCORES              # output columns per core = 512
NKT = H // 128                   # 32 k-tiles over the model dim
NR = QR // 128                   # 12 k-tiles over q_lora_rank
NKR = KR // 128                  # 4 k-tiles over kv_lora_rank
KD = KR + DR                     # 576
MASKV = -30000.0

LAST_RESULT = None               # test harness reads exec_time_ns from here
_CACHED_NC = None


def _build_program():
    nc = bacc.Bacc(None, target_bir_lowering=False, num_devices=NCORES)

    # ---------------- DRAM declarations ----------------
    xP = nc.dram_tensor("xP", [128, NKT * SL], F16, kind="ExternalInput")
    qawP = nc.dram_tensor("qawP", [12, 128, 8 * 512], F16, kind="ExternalInput")
    kvawP = nc.dram_tensor("kvawP", [4, 128, 8 * 576], F16, kind="ExternalInput")
    qbwP = nc.dram_tensor("qbwP", [NR, 128, 1024], F16, kind="ExternalInput")
    kvbwP = nc.dram_tensor("kvbwP", [NKR, 128, 1024], F16, kind="ExternalInput")
    owP = nc.dram_tensor("owP", [N_HEADS * DV, MCOLS], F16, kind="ExternalInput")
    cosqP = nc.dram_tensor("cosqP", [128, S], F, kind="ExternalInput")
    sinqP = nc.dram_tensor("sinqP", [128, S], F, kind="ExternalInput")
    coskP = nc.dram_tensor("coskP", [2, 128, 32], F, kind="ExternalInput")
    sinkP = nc.dram_tensor("sinkP", [2, 128, 32], F, kind="ExternalInput")
    identH = nc.dram_tensor("identH", [128, 128], F16, kind="ExternalInput")
    maskTP = nc.dram_tensor("maskTP", [128, 128], F16, kind="ExternalInput")
    selP = nc.dram_tensor("selP", [4, 128, 128], F, kind="ExternalInput")
    out_d = nc.dram_tensor("out", [MCOLS, S], F, kind="ExternalOutput")

    # latents gathered in [token, feature] layout; consumers transpose-read
    g1kv_src = nc.dram_tensor("g1kv_src", [SL, KD], F16)
    g1kv = nc.dram_tensor("g1kv", [NCORES, SL, KD], F16, addr_space="Shared")
    g1q_src = nc.dram_tensor("g1q_src", [SL, QR], F16)
    g1q = nc.dram_tensor("g1q", [NCORES, SL, QR], F16, addr_space="Shared")
    g2_src = [nc.dram_tensor(f"g2src{p}", [2, DV, S], F16) for p in range(2)]
    g2 = [
        nc.dram_tensor(f"g2_{p}", [NCORES, 2, DV, S], F16, addr_space="Shared")
        for p in range(2)
    ]
    RG = [list(range(NCORES))]
    g1kv_flat = g1kv.ap().rearrange("c s f -> (c s) f")
    g1q_flat = g1q.ap().rearrange("c s f -> (c s) f")

    with TileContext(nc) as tc, ExitStack() as ctx:
        persist = ctx.enter_context(tc.tile_pool(name="persist", bufs=1))
        ckvp = ctx.enter_context(tc.tile_pool(name="ckvp", bufs=1))



        # input activations + first weight tiles first — the first matmul
        # waits only on chunk 0; constants follow (not needed until rms)
        xtp = ExitStack()
        xtpool = xtp.enter_context(tc.tile_pool(name="xtp", bufs=1))
        xt = xtpool.tile([128, NKT, SL], F16, name="xt")
        for g in range(4):
            nc.sync.dma_start(
                xt[:, ds(8 * g, 8), :],
                xP.ap()[:, ds(8 * g * SL, 8 * SL)].rearrange(
                    "p (kt s) -> p kt s", kt=8
                ),
            )
        wp0_e = ExitStack()
        wp0 = wp0_e.enter_context(tc.tile_pool(name="wp0", bufs=4))
        kvw_tiles = []
        for ktg in range(4):
            w = wp0.tile([128, 8, 576], F16, tag="kvw", name=f"kvw{ktg}")
            nc.sync.dma_start(
                w[:], kvawP.ap()[ktg].rearrange("p (k n) -> p k n", k=8)
            )
            kvw_tiles.append(w)

        # ---------------- constants ----------------
        identh = persist.tile([128, 128], F16, name="c_identh")
        nc.sync.dma_start(identh[:], identH[:])
        maskT = persist.tile([128, 128], F16, name="c_maskT")
        nc.sync.dma_start(maskT[:], maskTP[:])
        ones_col = persist.tile([128, 1], F16, name="c_ones_col")
        nc.vector.memset(ones_col[:], 1.0)
        sel4 = []
        for qb in range(4):
            s = persist.tile([128, 128], R, name=f"c_sel{qb}")
            nc.sync.dma_start(s[:], selP.ap()[qb].bitcast(R))
            sel4.append(s)
        eps_t = persist.tile([128, 1], F, name="c_eps")
        nc.vector.memset(eps_t[:], EPS)
        sin_k = [persist.tile([128, 32], F, name=f"t_sink{st}") for st in range(2)]
        cos_k = [persist.tile([128, 32], F, name=f"t_cosk{st}") for st in range(2)]
        for st in range(2):
            nc.sync.dma_start(sin_k[st][:], sinkP.ap()[st])
            nc.sync.dma_start(cos_k[st][:], coskP.ap()[st])

        # persistent attention-phase state
        kx = persist.tile([128, S], F16, name="kx")        # [64:128] = k_pe
        kpe0 = persist.tile([64, S], F16, name="kpe0")     # k_pe for even heads
        qnope = [persist.tile([128, S], F16, name=f"qnope{h}") for h in range(HPC)]
        qfpe = [persist.tile([128, S], F16, name=f"qfpe{p}") for p in range(2)]
        kn_all = [persist.tile([128, S], F16, name=f"kn{h}") for h in range(HPC)]
        v_sb = persist.tile([128, S // 128, 512], F16, name="v_sb")

        # ---------------- phase 0: q_a / kv_a projections ----------------
        ctx0 = ExitStack()
        p0 = ctx0.enter_context(tc.tile_pool(name="p0", bufs=2))

        # --- kv_a: 512-col ckv chunk + 64-col k_pe chunk per st ---
        ctx0a = ExitStack()
        kv0_ps = ctx0a.enter_context(tc.tile_pool(name="kv0_ps", bufs=2, space="PSUM"))
        kv1_ps = ctx0a.enter_context(tc.tile_pool(name="kv1_ps", bufs=2, space="PSUM"))

        kvch = [
            kv0_ps.tile([128, 512], F, tag="kv0", name=f"kvps{st}") for st in range(2)
        ]
        kvpe = [
            kv1_ps.tile([128, 64], F, tag="kv1", name=f"kvpe{st}") for st in range(2)
        ]
        for ktg in range(4):
            w = kvw_tiles[ktg]
            for kk in range(8):
                kt = ktg * 8 + kk
                for st in range(2):
                    nc.tensor.matmul(
                        kvch[st][:], xt[:, kt, ts(st, 128)], w[:, kk, 0:512],
                        start=(kt == 0), stop=(kt == NKT - 1),
                    )
                    nc.tensor.matmul(
                        kvpe[st][:], xt[:, kt, ts(st, 128)], w[:, kk, 512:576],
                        start=(kt == 0), stop=(kt == NKT - 1),
                    )
        for st in range(2):
            acc0 = p0.tile([128, 1], F, tag="kvacc", name=f"kvacc{st}")
            scr = p0.tile([128, 512], F, tag="kvscr", name=f"kvscr{st}")
            nc.scalar.activation(scr[:], kvch[st][:], AF.Square, accum_out=acc0[:])
            stdv = p0.tile([128, 1], F, tag="kvstd", name=f"kvstd{st}")
            nc.scalar.activation(
                stdv[:], acc0[:], AF.Sqrt, bias=eps_t[:], scale=1.0 / KR
            )
            rinv = p0.tile([128, 1], F, tag="kvrinv", name=f"kvrinv{st}")
            nc.vector.reciprocal(rinv[:], stdv[:])
            ckvn = p0.tile([128, KR], F16, tag="ckvn", name=f"ckvn{st}")
            nc.vector.tensor_scalar_mul(ckvn[:], kvch[st][:], rinv[:])
            nc.gpsimd.dma_start(g1kv_src.ap()[ts(st, 128), 0:KR], ckvn[:])
            # rope k_pe: deinterleave even/odd from the 64-col chunk
            pe = kvpe[st][:].rearrange("p (d two) -> p two d", two=2)
            y1, y2 = pe[:, 0], pe[:, 1]
            kr_t = p0.tile([128, DR], F16, tag="kr", name=f"kr{st}")
            t1 = p0.tile([128, 32], F, tag="krt1", name=f"krt1_{st}")
            t2 = p0.tile([128, 32], F, tag="krt2", name=f"krt2_{st}")
            nc.vector.tensor_tensor(t1[:], y1, cos_k[st][:], ALU.mult)
            nc.vector.tensor_tensor(t2[:], y2, sin_k[st][:], ALU.mult)
            nc.vector.tensor_tensor(kr_t[:, 0:32], t1[:], t2[:], ALU.subtract)
            nc.vector.tensor_tensor(t1[:], y2, cos_k[st][:], ALU.mult)
            nc.vector.tensor_tensor(t2[:], y1, sin_k[st][:], ALU.mult)
            nc.vector.tensor_tensor(kr_t[:, 32:64], t1[:], t2[:], ALU.add)
            nc.gpsimd.dma_start(g1kv_src.ap()[ts(st, 128), KR:KD], kr_t[:])
        ctx0a.close()

        nc.gpsimd.collective_compute(
            "AllGather", ALU.bypass,
            ins=[g1kv_src.ap().opt()], outs=[g1kv.ap().opt()], replica_groups=RG,
        )

        # --- q_a (3 chunks of 512 cols) ---
        ctx0b = ExitStack()
        qa_ps = ctx0b.enter_context(tc.tile_pool(name="qa_ps", bufs=6, space="PSUM"))
        qch = [[None] * 3 for _ in range(2)]
        for ch in range(3):
            for st in range(2):
                qch[st][ch] = qa_ps.tile(
                    [128, 512], F, tag="qa_ps", name=f"qaps{st}_{ch}"
                )
        for ch in range(3):
            for ktg in range(4):
                w = wp0.tile([128, 8, 512], F16, tag="qaw", name=f"qaw{ch}_{ktg}")
                nc.sync.dma_start(
                    w[:], qawP.ap()[ch * 4 + ktg].rearrange("p (k n) -> p k n", k=8)
                )
                for kk in range(8):
                    kt = ktg * 8 + kk
                    for st in range(2):
                        nc.tensor.matmul(
                            qch[st][ch][:], xt[:, kt, ts(st, 128)], w[:, kk],
                            start=(kt == 0), stop=(kt == NKT - 1),
                        )
        for st in range(2):
            accs = []
            scr = p0.tile([128, 512], F, tag="qascr", name=f"qascr{st}")
            for ch in range(3):
                a = p0.tile([128, 1], F, tag="qaacc", name=f"qaacc{st}_{ch}")
                nc.scalar.activation(scr[:], qch[st][ch][:], AF.Square, accum_out=a[:])
                accs.append(a)
            nc.vector.tensor_tensor(accs[0][:], accs[0][:], accs[1][:], ALU.add)
            nc.vector.tensor_tensor(accs[0][:], accs[0][:], accs[2][:], ALU.add)
            stdv = p0.tile([128, 1], F, tag="qastd", name=f"qastd{st}")
            nc.scalar.activation(
                stdv[:], accs[0][:], AF.Sqrt, bias=eps_t[:], scale=1.0 / QR
            )
            rinv = p0.tile([128, 1], F, tag="qarinv", name=f"qarinv{st}")
            nc.vector.reciprocal(rinv[:], stdv[:])
            qn = p0.tile([128, QR], F16, tag="qn", name=f"qn{st}")
            for ch in range(3):
                nc.vector.tensor_scalar_mul(qn[:, ts(ch, 512)], qch[st][ch][:], rinv[:])
            nc.gpsimd.dma_start(g1q_src.ap()[ts(st, 128), :], qn[:])
        ctx0b.close()
        ctx0.close()
        wp0_e.close()
        xtp.close()

        nc.gpsimd.collective_compute(
            "AllGather", ALU.bypass,
            ins=[g1q_src.ap().opt()], outs=[g1q.ap().opt()], replica_groups=RG,
        )

        # o_proj weights / output accumulators (space freed by phase 0)
        persist2 = ctx.enter_context(tc.tile_pool(name="persist2", bufs=1))
        osb = [persist2.tile([128, 4, 512], F, name=f"osb{mt}") for mt in range(4)]
        ow_t = [
            [persist2.tile([128, MCOLS], F16, name=f"ow{g}_{r8}")
             for r8 in range(NCORES)]
            for g in range(HPC)
        ]

        # q_b inputs first (no gather dependency — keeps the ring flowing),
        # then transpose-reads of the gathered kv latent, split in halves so
        # decompression can start on the first tokens early
        ctx2 = ExitStack()
        qbwp = ctx2.enter_context(tc.tile_pool(name="qbwp", bufs=1))
        qrp = ctx2.enter_context(tc.tile_pool(name="qrp", bufs=12))
        ropep = ctx2.enter_context(tc.tile_pool(name="ropep", bufs=1))
        sinp = ctx2.enter_context(tc.tile_pool(name="sinp", bufs=1))
        qbw = []
        for r in range(NR):
            t = qbwp.tile([128, 1024], F16, name=f"qbw{r}")
            nc.sync.dma_start(t[:], qbwP.ap()[r])
            qbw.append(t)
        kvbw = []
        for r in range(NKR):
            t = ckvp.tile([128, 1024], F16, name=f"kvbw{r}")
            nc.sync.dma_start(t[:], kvbwP.ap()[r])
            kvbw.append(t)
        ckv_t = [ckvp.tile([128, S], F16, name=f"ckv{r}") for r in range(NKR)]
        for half in range(2):
            for r in range(NKR):
                nc.sync.dma_start_transpose(
                    ckv_t[r][:, ts(half, 1024)],
                    g1kv_flat[ds(1024 * half, 1024), ts(r, 128)],
                )
        nc.sync.dma_start_transpose(kx[:], g1kv_flat[:, KD - 128 : KD])
        nc.gpsimd.dma_start(kpe0[:], kx[ds(64, 64), :])

        # ---------------- decompression (overlaps the q AllGather) ----------
        dec_ctx = ExitStack()
        dec_ps = dec_ctx.enter_context(tc.tile_pool(name="dec_ps", bufs=4, space="PSUM"))
        for st in range(S // 128):
            ps = dec_ps.tile([128, 512], F, tag="dec", name=f"vps{st}")
            for r in range(NKR):
                nc.tensor.matmul(
                    ps[:], ckv_t[r][:, ts(st, 128)], kvbw[r][:, 512:1024],
                    start=(r == 0), stop=(r == NKR - 1),
                )
            nc.vector.tensor_copy(v_sb[:, st, :], ps[:])
        for h in range(HPC):
            for sb in range(4):
                ps = dec_ps.tile([128, 512], F, tag="dec", name=f"knps{h}_{sb}")
                for r in range(NKR):
                    nc.tensor.matmul(
                        ps[:], kvbw[r][:, ts(h, 128)], ckv_t[r][:, ts(sb, 512)],
                        start=(r == 0), stop=(r == NKR - 1),
                    )
                nc.scalar.copy(kn_all[h][:, ts(sb, 512)], ps[:])
        dec_ctx.close()

        # ---------------- q_b projection (+ q rope) ----------------
        qb_ps_ctx = ExitStack()
        qb_ps = qb_ps_ctx.enter_context(tc.tile_pool(name="qb_ps", bufs=8, space="PSUM"))

        for sb in range(4):
            sin_q = sinp.tile([128, 512], F, tag="sinq", name=f"t_sinq{sb}")
            cos_q = sinp.tile([128, 512], F, tag="cosq", name=f"t_cosq{sb}")
            nc.sync.dma_start(sin_q[:], sinqP.ap()[:, ts(sb, 512)])
            nc.sync.dma_start(cos_q[:], cosqP.ap()[:, ts(sb, 512)])
            qr_tiles = []
            for r in range(NR):
                t = qrp.tile([128, 512], F16, tag="qr", name=f"qr{sb}_{r}")
                nc.sync.dma_start_transpose(
                    t[:], g1q_flat[ds(2 * sb * SL, 2 * SL), ts(r, 128)]
                )
                qr_tiles.append(t)
            rope_t1 = {}
            for j in range(8):
                psj = qb_ps.tile([128, 512], F, tag="qb_ps", name=f"qbps{sb}_{j}")
                for r in range(NR):
                    nc.tensor.matmul(
                        psj[:], qbw[r][:, ts(j, 128)], qr_tiles[r][:],
                        start=(r == 0), stop=(r == NR - 1),
                    )
                if j < 4:
                    nc.scalar.copy(qnope[j][:, ts(sb, 512)], psj[:])
                elif j < 6:
                    p = j - 4
                    t1 = ropep.tile([128, 512], F, tag=f"rope1_{p}", name=f"rp1_{sb}_{p}")
                    nc.vector.tensor_tensor(t1[:], psj[:], cos_q[:], ALU.mult)
                    rope_t1[p] = t1
                else:
                    p = j - 6
                    t2 = ropep.tile([128, 512], F, tag=f"rope2_{p}", name=f"rp2_{sb}_{p}")
                    nc.vector.tensor_tensor(t2[:], psj[:], sin_q[:], ALU.mult)
                    nc.vector.tensor_tensor(
                        qfpe[p][:, ts(sb, 512)], rope_t1[p][:], t2[:], ALU.add
                    )
        qb_ps_ctx.close()
        ctx2.close()

        # o_proj weight prefetch (flows during attention head 0)
        for g in range(HPC):
            for r8 in range(NCORES):
                nc.sync.dma_start(
                    ow_t[g][r8][:], owP.ap()[ts(HPC * r8 + g, 128), :]
                )

        # ---------------- attention + interleaved o_proj ----------------
        ctx3 = ExitStack()
        probp = ctx3.enter_context(tc.tile_pool(name="probp", bufs=3))
        attup = ctx3.enter_context(tc.tile_pool(name="attup", bufs=4))
        attp = ctx3.enter_context(tc.tile_pool(name="attp", bufs=4))
        denp = ctx3.enter_context(tc.tile_pool(name="denp", bufs=2))
        g2hp = ctx3.enter_context(tc.tile_pool(name="g2hp", bufs=2))
        sc_ps = ctx3.enter_context(tc.tile_pool(name="sc_ps", bufs=3, space="PSUM"))
        av_ps = ctx3.enter_context(tc.tile_pool(name="av_ps", bufs=1, space="PSUM"))
        den_ps = ctx3.enter_context(tc.tile_pool(name="den_ps", bufs=1, space="PSUM"))
        bc_ps = ctx3.enter_context(tc.tile_pool(name="bc_ps", bufs=1, space="PSUM"))
        o_ps = ctx3.enter_context(tc.tile_pool(name="o_ps", bufs=2, space="PSUM"))

        def attention_head(h, flush_prev):
            """Emit head h. Returns a flush closure that emits the
            normalization + g2 writes + AllGather trigger for this head;
            the caller schedules it into a later phase so the PE never
            waits on the softmax reciprocals."""
            kn = kn_all[h]
            pe_rhs = qfpe[h // 2][ds(64 * (h % 2), 64), :]
            pe_lhs = kpe0 if h % 2 == 0 else kx[ds(64, 64), :]
            den_all = denp.tile([128, 512], F, tag="den_all", name=f"denall{h}")
            rinv_all = denp.tile([128, 512], R, tag="rinv_all", name=f"rinvall{h}")
            # unwritten rows are read (x0) by the selector matmul: keep finite
            nc.vector.memset(den_all[:], 1.0)
            attu = []
            for qb in range(4):
                nkt = 4 * (qb + 1)
                avt = av_ps.tile([128, 512], F, tag="av", name=f"av{h}_{qb}")
                dent = den_ps.tile([1, 512], F, tag="den", name=f"den{h}_{qb}")
                sct = [None] * nkt
                prob = [None] * nkt

                def emit_score(kt):
                    trim = max(0, 128 * (kt - 4 * qb))
                    qsl = ds(512 * qb + trim, 512 - trim)
                    sct[kt] = sc_ps.tile([128, 512], F, tag="sc", name=f"sc{h}{qb}_{kt}")
                    nc.tensor.matmul(
                        sct[kt][:, trim:512], kn[:, ts(kt, 128)], qnope[h][:, qsl],
                        start=True, stop=False,
                    )
                    nc.tensor.matmul(
                        sct[kt][:, trim:512], pe_lhs[:, ts(kt, 128)], pe_rhs[:, qsl],
                        start=False, stop=(kt < 4 * qb),
                    )
                    if kt >= 4 * qb:
                        nc.tensor.matmul(
                            sct[kt][:, trim : trim + 128], maskT[:], identh[:],
                            start=False, stop=True,
                        )
                    prob[kt] = probp.tile(
                        [128, 512], F16, tag="prob", name=f"pr{h}{qb}_{kt}"
                    )
                    nc.scalar.activation(
                        prob[kt][:, trim:512], sct[kt][:, trim:512], AF.Exp
                    )

                def emit_avden(kt):
                    trim = max(0, 128 * (kt - 4 * qb))
                    nc.tensor.matmul(
                        avt[:, trim:512], v_sb[:, kt, ts(h, 128)],
                        prob[kt][:, trim:512],
                        start=(kt == 0), stop=(kt == nkt - 1),
                    )
                    nc.tensor.matmul(
                        dent[:, trim:512], ones_col[:], prob[kt][:, trim:512],
                        start=(kt == 0), stop=(kt == nkt - 1),
                    )

                for kt in range(nkt):
                    emit_score(kt)
                    if qb == 1 and kt == 1 and flush_prev is not None:
                        flush_prev()
                        flush_prev = None
                    if kt > 0:
                        emit_avden(kt - 1)
                emit_avden(nkt - 1)
                nc.vector.tensor_copy(den_all[ds(32 * qb, 1), :], dent[:])
                at = attup.tile([128, 512], F, tag="attu", name=f"attu{h}_{qb}")
                nc.scalar.copy(at[:], avt[:])
                attu.append(at)
            with nc.allow_low_precision(reason="f32r out is fp32 storage"):
                nc.vector.reciprocal(rinv_all[:], den_all[:])
            if flush_prev is not None:
                flush_prev()

            def flush():
                for qb in range(4):
                    bc = bc_ps.tile([128, 512], F, tag="bc", name=f"bc{h}_{qb}")
                    nc.tensor.matmul(
                        bc[:], sel4[qb][:], rinv_all[:], start=True, stop=True,
                    )
                    att = attp.tile([128, 512], F16, tag="att", name=f"att{h}_{qb}")
                    nc.vector.tensor_tensor(att[:], attu[qb][:], bc[:], ALU.mult)
                    nc.gpsimd.dma_start(
                        g2_src[h // 2].ap()[h % 2, :, ts(qb, 512)], att[:]
                    )
                if h % 2 == 1:
                    p = h // 2
                    nc.gpsimd.collective_compute(
                        "AllGather", ALU.bypass,
                        ins=[g2_src[p].ap().opt()], outs=[g2[p].ap().opt()],
                        replica_groups=RG,
                    )

            return flush

        def oproj_group(g, flush_prev=None):
            for sg in range(4):  # 512-col seq block
                g2h = g2hp.tile([128, NCORES, 512], F16, tag="g2h", name=f"g2h{g}_{sg}")
                nc.sync.dma_start(
                    g2h[:],
                    g2[g // 2].ap()[:, g % 2, :, ts(sg, 512)]
                    .rearrange("c p s -> p c s"),
                )
                for mt in range(4):
                    ps = o_ps.tile([128, 512], F, tag="o", name=f"ops{g}_{sg}_{mt}")
                    for r8 in range(NCORES):
                        nc.tensor.matmul(
                            ps[:], ow_t[g][r8][:, ts(mt, 128)],
                            g2h[:, r8, :],
                            start=(r8 == 0), stop=(r8 == NCORES - 1),
                        )
                    if g == 0:
                        nc.scalar.copy(osb[mt][:, sg, :], ps[:])
                    else:
                        nc.vector.tensor_tensor(
                            osb[mt][:, sg, :], osb[mt][:, sg, :], ps[:], ALU.add
                        )
                    if flush_prev is not None and sg == 0 and mt == 1:
                        flush_prev()
                        flush_prev = None

        f0 = attention_head(0, None)
        f1 = attention_head(1, f0)
        f2 = attention_head(2, f1)
        f3 = attention_head(3, f2)
        oproj_group(0, flush_prev=f3)
        oproj_group(1)
        oproj_group(2)
        oproj_group(3)

        for mt in range(4):
            nc.gpsimd.dma_start(
                out_d.ap()[ts(mt, 128), :],
                osb[mt][:].rearrange("p sg c -> p (sg c)"),
            )
        ctx3.close()

    nc.compile()
    return nc


def _get_nc():
    global _CACHED_NC
    if _CACHED_NC is None:
        _CACHED_NC = _build_program()
    return _CACHED_NC


def _prep_inputs(hidden_states, position_ids, q_a_w, q_a_ln_w, q_b_w, kv_a_w,
                 kv_a_ln_w, kv_b_w, o_w):
    hidden_states = np.asarray(hidden_states, dtype=np.float32)
    position_ids = np.asarray(position_ids, dtype=np.int32)
    q_a_w = np.asarray(q_a_w, dtype=np.float32)
    q_a_ln_w = np.asarray(q_a_ln_w, dtype=np.float32)
    q_b_w = np.asarray(q_b_w, dtype=np.float32)
    kv_a_w = np.asarray(kv_a_w, dtype=np.float32)
    kv_a_ln_w = np.asarray(kv_a_ln_w, dtype=np.float32)
    kv_b_w = np.asarray(kv_b_w, dtype=np.float32)
    o_w = np.asarray(o_w, dtype=np.float32)

    xT_full = hidden_states[0].T                                 # (H, S)
    # weight tiles packed in exact kernel load order (contiguous DMA)
    qa = q_a_w.T.reshape(4, 8, 128, 3, 512)                      # (ktg,kk,p,ch,n)
    qawP = np.ascontiguousarray(
        qa.transpose(3, 0, 2, 1, 4).reshape(12, 128, 8 * 512)
    ).astype(np.float16)
    kva = kv_a_w.T.reshape(4, 8, 128, 576)                       # (ktg,kk,p,m)
    kvawP = np.ascontiguousarray(
        kva.transpose(0, 2, 1, 3).reshape(4, 128, 8 * 576)
    ).astype(np.float16)

    identH = np.eye(128, dtype=np.float16)
    maskT = np.zeros((128, 128), dtype=np.float32)
    maskT[np.triu_indices(128, 1)] = MASKV                       # cols > rows
    maskT = maskT.astype(np.float16)
    selP = np.zeros((4, 128, 128), dtype=np.float32)
    for qb in range(4):
        selP[qb, 32 * qb, :] = 1.0

    # rope tables from position_ids (host-computed)
    inv_freq = (1.0 / (THETA ** (np.arange(0, DR, 2) / DR))).astype(np.float64)
    pos = position_ids.reshape(-1).astype(np.float64)            # (S,)
    ang_q = pos[None, :] * np.tile(inv_freq, 4)[:, None]         # (128, S)
    cosqP = np.cos(ang_q).astype(np.float32)
    sinqP = np.sin(ang_q).astype(np.float32)

    owT_full = o_w.T                                             # (N*DV, H)

    in_maps = []
    for c in range(NCORES):
        heads = slice(HPC * c, HPC * (c + 1))
        qb = q_b_w.reshape(N_HEADS, QD, QR)[heads]               # (4, 192, QR)
        nope = qb[:, :DN, :].reshape(HPC * DN, QR)
        pe = qb[:, DN:, :]
        pe_d = np.concatenate([pe[:, 0::2, :], pe[:, 1::2, :]], axis=1)  # (4,64,QR)
        pe_r = np.concatenate([-pe_d[:, 32:, :], pe_d[:, :32, :]], axis=1)
        cols = np.concatenate(
            [nope, pe_d.reshape(HPC * DR, QR), pe_r.reshape(HPC * DR, QR)], axis=0
        )                                                        # (1024, QR)
        qbwT_c = (cols * (SCALE * q_a_ln_w[None, :])).T          # (QR, 1024)
        qbwP = np.ascontiguousarray(qbwT_c.reshape(NR, 128, 1024)).astype(np.float16)

        kvb = kv_b_w.reshape(N_HEADS, DN + DV, KR)[heads]
        kcols = np.concatenate(
            [kvb[:, :DN, :].reshape(HPC * DN, KR),
             kvb[:, DN:, :].reshape(HPC * DV, KR)],
            axis=0,
        )                                                        # (1024, KR)
        kvbwT_c = (kcols * kv_a_ln_w[None, :]).T                 # (KR, 1024)
        kvbwP = np.ascontiguousarray(kvbwT_c.reshape(NKR, 128, 1024)).astype(np.float16)

        # x tile: xP[p, kt*SL + s] = x[kt*128+p, SL*c + s]
        xs = xT_full[:, SL * c : SL * (c + 1)].reshape(NKT, 128, SL)
        xP = np.ascontiguousarray(xs.transpose(1, 0, 2).reshape(128, NKT * SL)
                                  ).astype(np.float16)

        # k-rope tables for this core's 256 tokens: [st, p, j]
        posc = pos[SL * c : SL * (c + 1)].reshape(2, 128)        # (st, p)
        ang_k = posc[:, :, None] * inv_freq[None, None, :]       # (2, 128, 32)
        coskP = np.cos(ang_k).astype(np.float32)
        sinkP = np.sin(ang_k).astype(np.float32)

        in_maps.append(
            {
                "xP": xP,
                "qawP": qawP,
                "kvawP": kvawP,
                "qbwP": qbwP,
                "kvbwP": kvbwP,
                "owP": np.ascontiguousarray(
                    owT_full[:, MCOLS * c : MCOLS * (c + 1)]
                ).astype(np.float16),
                "cosqP": cosqP,
                "sinqP": sinqP,
                "coskP": coskP,
                "sinkP": sinkP,
                "identH": identH,
                "maskTP": maskT,
                "selP": selP,
            }
        )
    return in_maps


def kernel(**inputs):
    global LAST_RESULT
    nc = _get_nc()
    in_maps = _prep_inputs(**inputs)
    res = run_bass_kernel_spmd(nc, in_maps, list(range(NCORES)))
    LAST_RESULT = res
    out = np.concatenate([res.results[c]["out"].T for c in range(NCORES)], axis=1)
    return out[None].astype(np.float32)
